# revision 12
# baseline (speedup 1.0000x reference)
"""Trainium2 Bass kernel for nn_CausalSelfAttention_5411658793445.

Sharding: queries (token dim) split 8 ways; K/V projection also token-split,
with the current block's roped K / V exchanged via ONE merged compact
AllGather so every core attends over the full kept KV window.

v3 structure:
  - order: K proj -> V proj -> merged K+V AllGather -> Q proj -> pass-1
    attention over prior KV (overlaps the AllGather) -> pass-2 over gathered
    current KV -> output projection.
  - prior K/V for the first 8 heads prefetched at t~0 (before the AllGather
    floods the shared DMA engines); weights streamed per-head chunk.
  - attention: l-tiles in groups of 3; one batched ACT exp per group
    (3-bank PSUM tile).  AV numerator+denominator via fused [V | ones]
    129-wide rhs into one packed PSUM bank [128, 3si, 130].  A single
    zeroing matmul opens the bank's accumulation group (start=True clears
    has_written bank-wide, so per-si interleaved starts would corrupt
    each other); all AV matmuls then accumulate with start=False.
  - 21+21 l-tiles (96 zero pads) instead of 45.
  - DVE offload: vt/part1 copies + one rope multiply run on GpSimd (Pool).
"""

import math
from contextlib import ExitStack

import numpy as np
import ml_dtypes

NC = 8
DIM, NH, HD = 1536, 12, 128
HALF = 64
H, W = 22, 40
FRAME = H * W            # 880
S_TOTAL = 2640
SC = S_TOTAL // NC       # 330
ST = 110                 # s-subtile (330 = 3*110)
NK = DIM // 128          # 12 contraction chunks
EPS = 1e-6
CT = 22
CH = 21
CW = 21
PREFETCH = 7             # prior-KV head pairs prefetched before the AllGather

_BF16 = ml_dtypes.bfloat16
_cache: dict = {}


def _build_theta(freqs_angle, cs):
    start_frame = cs // FRAME
    nf = S_TOTAL // FRAME
    t = freqs_angle[start_frame:start_frame + nf, :CT]
    h = freqs_angle[:H, CT:CT + CH]
    w = freqs_angle[:W, CT + CH:CT + CH + CW]
    tf = np.broadcast_to(t[:, None, None, :], (nf, H, W, CT))
    hf = np.broadcast_to(h[None, :, None, :], (nf, H, W, CH))
    wf = np.broadcast_to(w[None, None, :, :], (nf, H, W, CW))
    return np.concatenate([tf, hf, wf], axis=-1).reshape(nf * H * W, HALF)


def _segments(r0, r1):
    """Split row range [r0, r1) at 128 boundaries -> (a, b) pieces."""
    a = r0
    while a < r1:
        b = min(r1, (a // 128 + 1) * 128)
        yield a, b
        a = b


def _build_program(n_prior, npk, n_pads):
    import concourse.bass as bass  # noqa: F401
    import concourse.tile as tile
    from concourse import bacc, mybir
    from concourse.masks import make_identity

    f32 = mybir.dt.float32
    bf16 = mybir.dt.bfloat16
    Act = mybir.ActivationFunctionType
    Alu = mybir.AluOpType

    NTP = npk // 128                  # prior l-tiles (21)
    NCK = -(-S_TOTAL // 128) * 128    # current cols padded (2688)
    NTC = NCK // 128                  # current l-tiles (21)
    sm_scale = 1.0 / math.sqrt(HD)

    nc = bacc.Bacc("TRN2", target_bir_lowering=False, debug=False,
                   num_devices=NC)

    xT = nc.dram_tensor("xT", [DIM, SC], bf16, kind="ExternalInput").ap()
    thetaT = nc.dram_tensor("thetaT", [HALF, SC], f32, kind="ExternalInput").ap()
    wq = nc.dram_tensor("wq", [DIM, DIM], bf16, kind="ExternalInput").ap()
    wk = nc.dram_tensor("wk", [DIM, DIM], bf16, kind="ExternalInput").ap()
    wv = nc.dram_tensor("wv", [DIM, DIM], bf16, kind="ExternalInput").ap()
    wo = nc.dram_tensor("wo", [DIM, DIM], bf16, kind="ExternalInput").ap()
    bq2 = nc.dram_tensor("bq2", [HD, NH], f32, kind="ExternalInput").ap()
    bk2 = nc.dram_tensor("bk2", [HD, NH], f32, kind="ExternalInput").ap()
    gq2 = nc.dram_tensor("gq2", [HD, NH], f32, kind="ExternalInput").ap()
    gk2 = nc.dram_tensor("gk2", [HD, NH], f32, kind="ExternalInput").ap()
    bv1 = nc.dram_tensor("bv1", [1, DIM], bf16, kind="ExternalInput").ap()
    bo1 = nc.dram_tensor("bo1", [1, DIM], bf16, kind="ExternalInput").ap()
    pswT = nc.dram_tensor("pswT", [HD, HD], bf16, kind="ExternalInput").ap()
    priorKT = nc.dram_tensor("priorKT", [NH, HD, npk], bf16,
                             kind="ExternalInput").ap()
    # host-pretiled prior V: [h, p, t, d] = prior_kept[t*128+p, h, d]
    priorVT2 = nc.dram_tensor("priorVT2", [NH, 128, NTP, HD], bf16,
                              kind="ExternalInput").ap()
    out = nc.dram_tensor("out", [SC, DIM], f32, kind="ExternalOutput").ap()
    import os as _os
    _dbg = bool(int(_os.environ.get("KERNEL_DEBUG", "0")))
    if _dbg:
        dbg_part1 = nc.dram_tensor("dbg_part1", [128, NH, 3, 130], f32,
                                   kind="ExternalOutput").ap()
        dbg_kch = nc.dram_tensor("dbg_kch", [128, NCK], bf16,
                                 kind="ExternalOutput").ap()
        dbg_vch = nc.dram_tensor("dbg_vch", [128, NTC, 130], bf16,
                                 kind="ExternalOutput").ap()

    w_re = "(ko ki) m -> ki ko m"

    with tile.TileContext(nc, trace_sim=False) as tc, ExitStack() as ctx:
        consts = ctx.enter_context(tc.tile_pool(name="consts", bufs=1))
        smal = ctx.enter_context(tc.tile_pool(name="smal", bufs=4))
        sqp = ctx.enter_context(tc.tile_pool(name="sqp", bufs=2))
        csrp = ctx.enter_context(tc.tile_pool(name="csrp", bufs=2))
        outp = ctx.enter_context(tc.tile_pool(name="outp", bufs=1))
        wstr = ctx.enter_context(tc.tile_pool(name="wstr", bufs=3))
        kvp = ctx.enter_context(tc.tile_pool(name="kvp", bufs=PREFETCH))
        escp = ctx.enter_context(tc.tile_pool(name="escp", bufs=3))
        dram = ctx.enter_context(tc.tile_pool(name="dram", bufs=1, space="DRAM"))

        # ---------- constants ----------
        _constv_cache = {}

        def constv(val):
            if val not in _constv_cache:
                t = consts.tile([128, 1], f32, name=f"cv_{len(_constv_cache)}")
                nc.vector.memset(t, val)
                _constv_cache[val] = t
            return _constv_cache[val]

        ident = consts.tile([128, 128], f32)
        make_identity(nc, ident)
        ones_col = consts.tile([128, 1], f32)
        nc.vector.memset(ones_col, 1.0)
        ones_row = consts.tile([1, 128], bf16)
        nc.vector.memset(ones_row, 1.0)
        ones_row_f = consts.tile([1, 128], f32)
        nc.vector.memset(ones_row_f, 1.0)
        zero_col = consts.tile([1, 128], bf16)
        nc.vector.memset(zero_col, 0.0)
        zrow = consts.tile([1, 512], bf16)
        nc.vector.memset(zrow, 1.0)
        psw_sb = consts.tile([HD, HD], bf16)
        nc.sync.dma_start(psw_sb, pswT)
        th2 = consts.tile([128, SC], f32)
        nc.sync.dma_start(th2[0:HALF, :], thetaT)
        nc.sync.dma_start(th2[HALF:128, :], thetaT)
        # CC = [cos; cos], SS = [-sin; sin]
        cc = consts.tile([128, SC], f32)
        ss = consts.tile([128, SC], f32)
        nc.scalar.activation(cc, th2, Act.Sin, bias=constv(math.pi / 2.0))
        nc.scalar.activation(ss[0:HALF, :], th2[0:HALF, :], Act.Sin,
                             scale=constv(-1.0)[0:HALF])
        nc.scalar.activation(ss[HALF:128, :], th2[HALF:128, :], Act.Sin)
        bq_sb = consts.tile([HD, NH], f32)
        bk_sb = consts.tile([HD, NH], f32)
        gq_sb = consts.tile([HD, NH], f32)
        gk_sb = consts.tile([HD, NH], f32)
        nc.sync.dma_start(bq_sb, bq2)
        nc.sync.dma_start(bk_sb, bk2)
        nc.sync.dma_start(gq_sb, gq2)
        nc.sync.dma_start(gk_sb, gk2)
        bqg = consts.tile([HD, NH], f32)
        bkg = consts.tile([HD, NH], f32)
        nc.vector.tensor_mul(bqg, bq_sb, gq_sb)
        nc.vector.tensor_mul(bkg, bk_sb, gk_sb)
        bv_sb = consts.tile([1, DIM], bf16)
        bo_sb = consts.tile([1, DIM], bf16)
        nc.sync.dma_start(bv_sb, bv1)
        nc.sync.dma_start(bo_sb, bo1)

        # ---------- internal DRAM for the merged collective ----------
        kv_cc_in = dram.tile([2, NH, SC * HD], bf16)
        kvg = dram.tile([NC, 2, NH, SC * HD], bf16, addr_space="Shared")
        rgroups = [list(range(NC))]

        # ---------- prior-KV prefetch (before any collective traffic) -----
        def load_prior(h):
            pkh = kvp.tile([128, npk], bf16, tag="kload", name=f"pk_{h}")
            nc.sync.dma_start(pkh, priorKT[h])
            pvh = kvp.tile([128, NTP, 130], bf16, tag="vload", name=f"pv_{h}")
            nc.sync.dma_start(pvh[:, :, 0:HD], priorVT2[h])
            nc.vector.memset(pvh[:, :, 128:129], 1.0)
            return pkh, pvh

        prior_tiles = {}
        for h in range(min(PREFETCH, NH)):
            prior_tiles[h] = load_prior(h)

        # ================= phase P: projections (inner pools) =============
        pctx = ExitStack()
        acts = pctx.enter_context(tc.tile_pool(name="acts", bufs=1))
        knp = pctx.enter_context(tc.tile_pool(name="knp", bufs=2))
        ppp = pctx.enter_context(tc.tile_pool(name="ppp", bufs=2, space="PSUM"))
        pssp = pctx.enter_context(tc.tile_pool(name="pssp", bufs=1, space="PSUM"))
        pswp = pctx.enter_context(tc.tile_pool(name="pswp", bufs=2, space="PSUM"))

        xs = acts.tile([128, NK, SC], bf16)
        nc.sync.dma_start(xs, xT.rearrange("(ko ki) t -> ki ko t", ki=128))

        def qk_projection(w_dram, b_sb, g_sb, bg_sb, name):
            raw = acts.tile([128, NH, SC], bf16, tag="raw", name=f"raw_{name}")
            pss = pssp.tile([128, 512], f32, tag="pss", name=f"pss_{name}")
            for m in range(NH):
                wm = wstr.tile([128, NK, 128], bf16, tag="wm",
                               name=f"wm_{name}_{m}")
                nc.sync.dma_start(
                    wm, w_dram.rearrange(w_re, ki=128)[:, :, m * 128:(m + 1) * 128])
                ps = ppp.tile([128, 512], f32, tag="pp", name=f"pj_{name}_{m}")
                for kk in range(NK):
                    nc.tensor.matmul(
                        ps[:, :SC], wm[:, kk, :], xs[:, kk, :],
                        start=(kk == 0), stop=(kk == NK - 1))
                nc.scalar.activation(raw[:, m, :], ps[:, :SC], Act.Identity,
                                     bias=bg_sb[:, m:m + 1],
                                     scale=g_sb[:, m:m + 1])
                sq = sqp.tile([128, SC], f32, tag="sq")
                nc.scalar.activation(sq, ps[:, :SC], Act.Square,
                                     bias=b_sb[:, m:m + 1])
                nc.tensor.matmul(pss[0:1, :SC], ones_col, sq,
                                 start=(m == 0), stop=(m == NH - 1))
            r1 = smal.tile([1, SC], f32, tag="r1")
            nc.scalar.activation(r1, pss[0:1, :SC], Act.Sqrt,
                                 scale=constv(1.0 / DIM)[0:1],
                                 bias=constv(EPS)[0:1])
            rr = smal.tile([1, SC], f32, tag="rr")
            nc.vector.reciprocal(rr, r1)
            rrb = pswp.tile([128, 512], f32, tag="psw", name=f"rrb_{name}")
            nc.tensor.matmul(rrb[:, :SC], ones_row_f, rr, start=True, stop=True)
            ccr = csrp.tile([128, SC], f32, tag="ccr")
            ssr = csrp.tile([128, SC], f32, tag="ssr")
            nc.vector.tensor_mul(ccr, cc, rrb[:, :SC])
            nc.vector.tensor_mul(ssr, ss, rrb[:, :SC])
            return raw, ccr, ssr

        def rope_chunk(raw, ccr, ssr, m, dst_ap, name):
            # dst = raw*ccr + swap_halves(raw)*ssr   (swap via PE matmul)
            pw = pswp.tile([128, 512], f32, tag="psw", name=f"sw_{name}_{m}")
            nc.tensor.matmul(pw[:, :SC], psw_sb, raw[:, m, :],
                             start=True, stop=True)
            m1 = sqp.tile([128, SC], f32, tag="m1")
            nc.gpsimd.tensor_mul(m1, raw[:, m, :], ccr)
            m2 = sqp.tile([128, SC], f32, tag="m2")
            nc.vector.tensor_mul(m2, pw[:, :SC], ssr)
            nc.gpsimd.tensor_add(dst_ap, m1, m2)

        # ---------- K ----------
        raw_k, ccr_k, ssr_k = qk_projection(wk, bk_sb, gk_sb, bkg, "k")
        for m in range(NH):
            kn = knp.tile([128, SC], bf16, tag="kn", name=f"kn_{m}")
            rope_chunk(raw_k, ccr_k, ssr_k, m, kn, "k")
            nc.sync.dma_start(
                kv_cc_in[0, m].rearrange("(d t) -> d t", d=HD), kn)

        # ---------- V (direct [t, d] production, sequential tci) ----------
        vt = acts.tile([128, 3, DIM], bf16)
        for oc in range(3):
            wc = wstr.tile([128, NK, 512], bf16, tag="wc", name=f"wv_{oc}")
            nc.sync.dma_start(
                wc, wv.rearrange(w_re, ki=128)[:, :, oc * 512:(oc + 1) * 512])
            for tci in range(3):
                pv = ppp.tile([128, 512], f32, tag="pp", name=f"pv_{oc}_{tci}")
                for kk in range(NK):
                    nc.tensor.matmul(
                        pv[:ST, :], xs[:, kk, tci * ST:(tci + 1) * ST],
                        wc[:, kk, :], start=(kk == 0), stop=False)
                nc.tensor.matmul(
                    pv[:ST, :], ones_row[:, :ST],
                    bv_sb[:, oc * 512:(oc + 1) * 512],
                    start=False, stop=True)
                nc.vector.tensor_copy(
                    vt[:ST, tci, oc * 512:(oc + 1) * 512], pv[:ST, :])
        for h in range(NH):
            nc.sync.dma_start(
                kv_cc_in[1, h].rearrange("(tc p d) -> p tc d",
                                         tc=3, p=ST, d=HD),
                vt[:ST, :, h * HD:(h + 1) * HD])
        nc.gpsimd.collective_compute(
            "AllGather", Alu.bypass, replica_groups=rgroups,
            ins=[kv_cc_in.opt()], outs=[kvg.opt()])

        # ---------- Q ----------
        raw_q, ccr_q, ssr_q = qk_projection(wq, bq_sb, gq_sb, bqg, "q")
        qn = outp.tile([128, NH, SC], bf16)
        for m in range(NH):
            rope_chunk(raw_q, ccr_q, ssr_q, m, qn[:, m, :], "q")

        pctx.close()   # free xs/raw/vt SBUF + projection PSUM

        # ================= phase A: attention =============================
        wpool2 = ctx.enter_context(tc.tile_pool(name="wpool2", bufs=1))
        grp = ctx.enter_context(tc.tile_pool(name="grp", bufs=2, space="PSUM"))
        posp = ctx.enter_context(tc.tile_pool(name="posp", bufs=2, space="PSUM"))

        part1 = wpool2.tile([128, NH, 3, 130], f32)
        oT = wpool2.tile([128, NH, SC], bf16)
        smv = constv(sm_scale)

        def attn_pass(h, kt, vt_t, ntiles, phase):
            pos = posp.tile([128, 3, 130], f32, tag="pos",
                            name=f"pos_{phase}_{h}")
            # open the bank's single accumulation group (bank-wide bit clear)
            nc.tensor.matmul(pos[:, :, :], zero_col, zrow[:, 0:390],
                             start=True, stop=False, skip_group_check=True)
            for g0 in range(0, ntiles, 3):
                gs = min(3, ntiles - g0)
                gp = grp.tile([128, 3, 512], f32, tag="grp",
                              name=f"g_{phase}_{h}_{g0}")
                for j in range(gs):
                    lt = g0 + j
                    nc.tensor.matmul(
                        gp[:, j, :SC], kt[:, lt * 128:(lt + 1) * 128],
                        qn[:, h, :], start=True, stop=True)
                esc = escp.tile([128, 3, SC], bf16, tag="esc")
                nc.scalar.activation(esc[:, :gs, :], gp[:, :gs, :SC],
                                     Act.Exp, scale=smv)
                for j in range(gs):
                    lt = g0 + j
                    for si in range(3):
                        nc.tensor.matmul(
                            pos[:ST, si, 0:129],
                            esc[:, j, si * ST:(si + 1) * ST],
                            vt_t[:, lt, 0:129],
                            start=False,
                            stop=(lt == ntiles - 1 and si == 2),
                            skip_group_check=True)
            return pos

        # ---- pass 1: prior KV (overlaps the AllGather) ----
        for h in range(NH):
            pkh, pvh = prior_tiles[h] if h in prior_tiles else load_prior(h)
            pos = attn_pass(h, pkh, pvh, NTP, "p")
            nc.vector.tensor_copy(part1[:ST, h, :, 0:129], pos[:ST, :, 0:129])
        if _dbg:
            nc.sync.dma_start(dbg_part1, part1)

        # ---- pass 2: gathered current KV ----
        for h in range(NH):
            kch = kvp.tile([128, NCK], bf16, tag="kload", name=f"kc_{h}")
            for c in range(NC):
                nc.sync.dma_start(
                    kch[:, c * SC:(c + 1) * SC],
                    kvg[c, 0, h].rearrange("(p t) -> p t", p=HD))
            if NCK > S_TOTAL:
                nc.vector.memset(kch[:, S_TOTAL:NCK], 0.0)
            vch = kvp.tile([128, NTC, 130], bf16, tag="vload", name=f"vc_{h}")
            if NCK > S_TOTAL:
                # pad rows live in the last tile; zero it before the row DMAs
                nc.vector.memset(vch[:, NTC - 1, 0:HD], 0.0)
            for c in range(NC):
                r0 = c * SC
                src2d = kvg[c, 1, h].rearrange("(s d) -> s d", s=SC, d=HD)
                for a, b in _segments(r0, r0 + SC):
                    nc.sync.dma_start(
                        vch[a % 128:a % 128 + (b - a), a // 128, 0:HD],
                        src2d[a - r0:b - r0, :])
            nc.vector.memset(vch[:, :, 128:129], 1.0)
            if _dbg and h == 0:
                nc.sync.dma_start(dbg_kch, kch)
                nc.sync.dma_start(dbg_vch, vch)
            pos = attn_pass(h, kch, vch, NTC, "c")

            # finalize head: num/den merge, divide, transpose to [d, t]
            tmp = sqp.tile([128, 3, 130], f32, tag="tmp", name=f"tmp_{h}")
            nc.vector.tensor_add(tmp[:ST, :, 0:129], pos[:ST, :, 0:129],
                                 part1[:ST, h, :, 0:129])
            den = smal.tile([128, 3, 1], f32, tag="den")
            nc.vector.tensor_scalar_add(den[:ST, :, :], tmp[:ST, :, 128:129],
                                        -float(n_pads))
            rcp = smal.tile([128, 3, 1], f32, tag="rcp")
            nc.vector.reciprocal(rcp[:ST, :, :], den[:ST, :, :])
            odv = sqp.tile([128, 3, 128], f32, tag="odv", name=f"odv_{h}")
            for si in range(3):
                nc.vector.tensor_scalar_mul(odv[:ST, si, :],
                                            tmp[:ST, si, 0:128],
                                            rcp[:ST, si, 0:1])
            ptr = posp.tile([128, 512], f32, tag="pos", name=f"ptr_{h}")
            for si in range(3):
                nc.tensor.transpose(ptr[:, si * ST:(si + 1) * ST],
                                    odv[:ST, si, :], ident[:ST, :ST])
            nc.vector.tensor_copy(oT[:, h, :], ptr[:, 0:SC])

        # ---------- output projection (streamed wo, psum -> DRAM direct) --
        for oc in range(3):
            woc = wstr.tile([128, NK, 512], bf16, tag="wc", name=f"wo_{oc}")
            nc.sync.dma_start(
                woc, wo.rearrange(w_re, ki=128)[:, :, oc * 512:(oc + 1) * 512])
            for tci in range(3):
                po = posp.tile([128, 512], f32, tag="pos",
                               name=f"po_{oc}_{tci}")
                for hh in range(NH):
                    nc.tensor.matmul(
                        po[:ST, :], oT[:, hh, tci * ST:(tci + 1) * ST],
                        woc[:, hh, :], start=(hh == 0), stop=False)
                nc.tensor.matmul(
                    po[:ST, :], ones_row[:, :ST],
                    bo_sb[:, oc * 512:(oc + 1) * 512],
                    start=False, stop=True)
                ob = sqp.tile([128, 512], f32, tag="ob", name=f"ob_{oc}_{tci}")
                nc.vector.tensor_copy(ob[:ST, :], po[:ST, :])
                nc.sync.dma_start(
                    out[tci * ST:(tci + 1) * ST, oc * 512:(oc + 1) * 512],
                    ob[:ST, :])

    nc.compile()
    return nc


def _prep(inputs):
    x = np.asarray(inputs["x"], np.float32)
    freqs_angle = np.asarray(inputs["freqs_angle"], np.float32)
    prior_k = np.asarray(inputs["prior_k"], np.float32)
    prior_v = np.asarray(inputs["prior_v"], np.float32)
    cs = int(np.asarray(inputs["current_start"]))

    block = 3 * FRAME
    block_end = (cs // block + 1) * block
    keep_from = max(0, block_end - 6 * FRAME)
    keep_size = min(cs + S_TOTAL - keep_from, prior_k.shape[0] + S_TOTAL)
    n_prior = keep_size - S_TOTAL
    p0 = prior_k.shape[0] - n_prior
    npk = -(-n_prior // 128) * 128
    nck = -(-S_TOTAL // 128) * 128
    n_pads = (npk - n_prior) + (nck - S_TOTAL)

    perm = np.concatenate(
        [h * HD + np.concatenate([np.arange(0, HD, 2), np.arange(1, HD, 2)])
         for h in range(NH)])

    WqT = np.ascontiguousarray(np.asarray(inputs["Wq"], np.float32)[perm].T).astype(_BF16)
    WkT = np.ascontiguousarray(np.asarray(inputs["Wk"], np.float32)[perm].T).astype(_BF16)
    WvT = np.ascontiguousarray(np.asarray(inputs["Wv"], np.float32).T).astype(_BF16)
    WoT = np.ascontiguousarray(np.asarray(inputs["Wo"], np.float32).T).astype(_BF16)

    def two(vec, p=None):
        v = np.asarray(vec, np.float32)
        if p is not None:
            v = v[p]
        return np.ascontiguousarray(v.reshape(NH, HD).T)

    bq2 = two(inputs["bq"], perm)
    bk2 = two(inputs["bk"], perm)
    gq2 = two(inputs["gq"], perm)
    gk2 = two(inputs["gk"], perm)
    bv1 = np.asarray(inputs["bv"], np.float32).reshape(1, DIM).astype(_BF16)
    bo1 = np.asarray(inputs["bo"], np.float32).reshape(1, DIM).astype(_BF16)

    pswT = np.zeros((HD, HD), _BF16)
    for r in range(HD):
        pswT[(r + HALF) % HD, r] = 1.0   # lhsT of the half-swap permutation

    theta = _build_theta(freqs_angle, cs)          # [S, 64]
    thetaT = np.ascontiguousarray(theta.T)

    pk = prior_k[p0:].reshape(n_prior, DIM)[:, perm]
    priorKT = np.zeros((DIM, npk), np.float32)
    priorKT[:, :n_prior] = pk.T
    priorKT = np.ascontiguousarray(priorKT.reshape(NH, HD, npk)).astype(_BF16)

    ntp = npk // 128
    buf = np.zeros((npk, NH, HD), np.float32)
    buf[:n_prior] = prior_v[p0:]
    priorVT2 = np.ascontiguousarray(
        buf.reshape(ntp, 128, NH, HD).transpose(2, 1, 0, 3)).astype(_BF16)

    xT = np.ascontiguousarray(x[0].T).astype(_BF16)              # [DIM, S]

    shared = dict(wq=WqT, wk=WkT, wv=WvT, wo=WoT, bq2=bq2, bk2=bk2,
                  gq2=gq2, gk2=gk2, bv1=bv1, bo1=bo1, pswT=pswT,
                  priorKT=priorKT, priorVT2=priorVT2)
    in_maps = []
    for c in range(NC):
        m = dict(shared)
        m["xT"] = np.ascontiguousarray(xT[:, c * SC:(c + 1) * SC])
        m["thetaT"] = np.ascontiguousarray(thetaT[:, c * SC:(c + 1) * SC])
        in_maps.append(m)
    return in_maps, (n_prior, npk, n_pads)


def kernel(**inputs) -> np.ndarray:
    import os
    from concourse.bass_utils import run_bass_kernel_spmd

    in_maps, key = _prep(inputs)
    if key not in _cache:
        _cache[key] = _build_program(*key)
    nc = _cache[key]

    trace = bool(int(os.environ.get("KERNEL_TRACE", "0")))
    try:
        res = run_bass_kernel_spmd(
            nc, in_maps, core_ids=list(range(NC)), trace=trace,
            trace_cores=list(range(NC)) if trace else None)
    except ModuleNotFoundError:
        res = run_bass_kernel_spmd(nc, in_maps, core_ids=list(range(NC)),
                                   trace=False)
    kernel.last_results = res
    outp = np.concatenate([res.results[c]["out"] for c in range(NC)], axis=0)
    return outp[None].astype(np.float32)


# revision 13
# speedup vs baseline: 1.0835x; 1.0835x over previous
"""Trainium2 Bass kernel for nn_CausalSelfAttention_5411658793445.

Sharding: queries (token dim) split 8 ways; K/V projection also token-split,
with the current block's roped K / V exchanged via ONE merged compact
AllGather so every core attends over the full kept KV window.

v3 structure:
  - order: K proj -> V proj -> merged K+V AllGather -> Q proj -> pass-1
    attention over prior KV (overlaps the AllGather) -> pass-2 over gathered
    current KV -> output projection.
  - prior K/V for the first 8 heads prefetched at t~0 (before the AllGather
    floods the shared DMA engines); weights streamed per-head chunk.
  - attention: l-tiles in groups of 3; one batched ACT exp per group
    (3-bank PSUM tile).  AV numerator+denominator via fused [V | ones]
    129-wide rhs into one packed PSUM bank [128, 3si, 130].  A single
    zeroing matmul opens the bank's accumulation group (start=True clears
    has_written bank-wide, so per-si interleaved starts would corrupt
    each other); all AV matmuls then accumulate with start=False.
  - 21+21 l-tiles (96 zero pads) instead of 45.
  - DVE offload: vt/part1 copies + one rope multiply run on GpSimd (Pool).
"""

import math
from contextlib import ExitStack

import numpy as np
import ml_dtypes

NC = 8
DIM, NH, HD = 1536, 12, 128
HALF = 64
H, W = 22, 40
FRAME = H * W            # 880
S_TOTAL = 2640
SC = S_TOTAL // NC       # 330
ST = 110                 # s-subtile (330 = 3*110)
NK = DIM // 128          # 12 contraction chunks
EPS = 1e-6
CT = 22
CH = 21
CW = 21
PREFETCH = 7             # prior-KV head pairs prefetched before the AllGather

_BF16 = ml_dtypes.bfloat16
_cache: dict = {}


def _build_theta(freqs_angle, cs):
    start_frame = cs // FRAME
    nf = S_TOTAL // FRAME
    t = freqs_angle[start_frame:start_frame + nf, :CT]
    h = freqs_angle[:H, CT:CT + CH]
    w = freqs_angle[:W, CT + CH:CT + CH + CW]
    tf = np.broadcast_to(t[:, None, None, :], (nf, H, W, CT))
    hf = np.broadcast_to(h[None, :, None, :], (nf, H, W, CH))
    wf = np.broadcast_to(w[None, None, :, :], (nf, H, W, CW))
    return np.concatenate([tf, hf, wf], axis=-1).reshape(nf * H * W, HALF)


def _segments(r0, r1):
    """Split row range [r0, r1) at 128 boundaries -> (a, b) pieces."""
    a = r0
    while a < r1:
        b = min(r1, (a // 128 + 1) * 128)
        yield a, b
        a = b


def _build_program(n_prior, npk, n_pads):
    import concourse.bass as bass  # noqa: F401
    import concourse.tile as tile
    from concourse import bacc, mybir
    from concourse.masks import make_identity

    f32 = mybir.dt.float32
    bf16 = mybir.dt.bfloat16
    Act = mybir.ActivationFunctionType
    Alu = mybir.AluOpType

    NTP = npk // 128                  # prior l-tiles (21)
    NCK = -(-S_TOTAL // 128) * 128    # current cols padded (2688)
    NTC = NCK // 128                  # current l-tiles (21)
    sm_scale = 1.0 / math.sqrt(HD)

    nc = bacc.Bacc("TRN2", target_bir_lowering=False, debug=False,
                   num_devices=NC)

    xT = nc.dram_tensor("xT", [DIM, SC], bf16, kind="ExternalInput").ap()
    thetaT = nc.dram_tensor("thetaT", [HALF, SC], f32, kind="ExternalInput").ap()
    wq = nc.dram_tensor("wq", [DIM, DIM], bf16, kind="ExternalInput").ap()
    wk = nc.dram_tensor("wk", [DIM, DIM], bf16, kind="ExternalInput").ap()
    wv = nc.dram_tensor("wv", [DIM, DIM], bf16, kind="ExternalInput").ap()
    wo = nc.dram_tensor("wo", [DIM, DIM], bf16, kind="ExternalInput").ap()
    bq2 = nc.dram_tensor("bq2", [HD, NH], f32, kind="ExternalInput").ap()
    bk2 = nc.dram_tensor("bk2", [HD, NH], f32, kind="ExternalInput").ap()
    gq2 = nc.dram_tensor("gq2", [HD, NH], f32, kind="ExternalInput").ap()
    gk2 = nc.dram_tensor("gk2", [HD, NH], f32, kind="ExternalInput").ap()
    bv1 = nc.dram_tensor("bv1", [1, DIM], bf16, kind="ExternalInput").ap()
    bo1 = nc.dram_tensor("bo1", [1, DIM], bf16, kind="ExternalInput").ap()
    pswT = nc.dram_tensor("pswT", [HD, HD], bf16, kind="ExternalInput").ap()
    priorKT = nc.dram_tensor("priorKT", [NH, HD, npk], bf16,
                             kind="ExternalInput").ap()
    # host-pretiled prior V: [h, p, t, d] = prior_kept[t*128+p, h, d]
    priorVT2 = nc.dram_tensor("priorVT2", [NH, 128, NTP, HD], bf16,
                              kind="ExternalInput").ap()
    out = nc.dram_tensor("out", [SC, DIM], f32, kind="ExternalOutput").ap()
    import os as _os
    _dbg = bool(int(_os.environ.get("KERNEL_DEBUG", "0")))
    if _dbg:
        dbg_part1 = nc.dram_tensor("dbg_part1", [128, NH, 3, 130], f32,
                                   kind="ExternalOutput").ap()
        dbg_kch = nc.dram_tensor("dbg_kch", [128, NCK], bf16,
                                 kind="ExternalOutput").ap()
        dbg_vch = nc.dram_tensor("dbg_vch", [128, NTC, 130], bf16,
                                 kind="ExternalOutput").ap()

    w_re = "(ko ki) m -> ki ko m"

    with tile.TileContext(nc, trace_sim=False) as tc, ExitStack() as ctx:
        consts = ctx.enter_context(tc.tile_pool(name="consts", bufs=1))
        smal = ctx.enter_context(tc.tile_pool(name="smal", bufs=4))
        sqp = ctx.enter_context(tc.tile_pool(name="sqp", bufs=2))
        csrp = ctx.enter_context(tc.tile_pool(name="csrp", bufs=2))
        outp = ctx.enter_context(tc.tile_pool(name="outp", bufs=1))
        wstr = ctx.enter_context(tc.tile_pool(name="wstr", bufs=3))
        kvp = ctx.enter_context(tc.tile_pool(name="kvp", bufs=PREFETCH))
        escp = ctx.enter_context(tc.tile_pool(name="escp", bufs=3))
        dram = ctx.enter_context(tc.tile_pool(name="dram", bufs=1, space="DRAM"))

        # ---------- constants ----------
        _constv_cache = {}

        def constv(val):
            if val not in _constv_cache:
                t = consts.tile([128, 1], f32, name=f"cv_{len(_constv_cache)}")
                nc.vector.memset(t, val)
                _constv_cache[val] = t
            return _constv_cache[val]

        ident = consts.tile([128, 128], f32)
        make_identity(nc, ident)
        ones_col = consts.tile([128, 1], f32)
        nc.vector.memset(ones_col, 1.0)
        ones_row = consts.tile([1, 128], bf16)
        nc.vector.memset(ones_row, 1.0)
        ones_row_f = consts.tile([1, 128], f32)
        nc.vector.memset(ones_row_f, 1.0)
        zero_col = consts.tile([1, 128], bf16)
        nc.vector.memset(zero_col, 0.0)
        zrow = consts.tile([1, 512], bf16)
        nc.vector.memset(zrow, 1.0)
        psw_sb = consts.tile([HD, HD], bf16)
        nc.sync.dma_start(psw_sb, pswT)
        th2 = consts.tile([128, SC], f32)
        nc.sync.dma_start(th2[0:HALF, :], thetaT)
        nc.sync.dma_start(th2[HALF:128, :], thetaT)
        # CC = [cos; cos], SS = [-sin; sin]
        cc = consts.tile([128, SC], f32)
        ss = consts.tile([128, SC], f32)
        nc.scalar.activation(cc, th2, Act.Sin, bias=constv(math.pi / 2.0))
        nc.scalar.activation(ss[0:HALF, :], th2[0:HALF, :], Act.Sin,
                             scale=constv(-1.0)[0:HALF])
        nc.scalar.activation(ss[HALF:128, :], th2[HALF:128, :], Act.Sin)
        bq_sb = consts.tile([HD, NH], f32)
        bk_sb = consts.tile([HD, NH], f32)
        gq_sb = consts.tile([HD, NH], f32)
        gk_sb = consts.tile([HD, NH], f32)
        nc.sync.dma_start(bq_sb, bq2)
        nc.sync.dma_start(bk_sb, bk2)
        nc.sync.dma_start(gq_sb, gq2)
        nc.sync.dma_start(gk_sb, gk2)
        bqg = consts.tile([HD, NH], f32)
        bkg = consts.tile([HD, NH], f32)
        nc.vector.tensor_mul(bqg, bq_sb, gq_sb)
        nc.vector.tensor_mul(bkg, bk_sb, gk_sb)
        bv_sb = consts.tile([1, DIM], bf16)
        bo_sb = consts.tile([1, DIM], bf16)
        nc.sync.dma_start(bv_sb, bv1)
        nc.sync.dma_start(bo_sb, bo1)

        # ---------- internal DRAM for the merged collective ----------
        kv_cc_in = dram.tile([2, NH, SC * HD], bf16)
        kvg = dram.tile([NC, 2, NH, SC * HD], bf16, addr_space="Shared")
        rgroups = [list(range(NC))]

        # ---------- prior-KV prefetch (before any collective traffic) -----
        def load_prior(h, startup):
            pkh = kvp.tile([128, npk], bf16, tag="kload", name=f"pk_{h}")
            nc.sync.dma_start(pkh, priorKT[h])
            pvh = kvp.tile([128, NTP, 130], bf16, tag="vload", name=f"pv_{h}")
            if startup:
                # descriptor-heavy strided dest: keep it off the sync queue
                nc.scalar.dma_start(pvh[:, :, 0:HD], priorVT2[h])
            else:
                half = NTP // 3
                nc.scalar.dma_start(pvh[:, 0:half, 0:HD],
                                    priorVT2[h][:, 0:half, :])
                nc.sync.dma_start(pvh[:, half:NTP, 0:HD],
                                  priorVT2[h][:, half:NTP, :])
            nc.vector.memset(pvh[:, :, 128:129], 1.0)
            return pkh, pvh

        prior_tiles = {}
        for h in range(min(PREFETCH, NH)):
            prior_tiles[h] = load_prior(h, True)

        # ================= phase P: projections (inner pools) =============
        pctx = ExitStack()
        acts = pctx.enter_context(tc.tile_pool(name="acts", bufs=1))
        knp = pctx.enter_context(tc.tile_pool(name="knp", bufs=2))
        ppp = pctx.enter_context(tc.tile_pool(name="ppp", bufs=2, space="PSUM"))
        pssp = pctx.enter_context(tc.tile_pool(name="pssp", bufs=1, space="PSUM"))
        pswp = pctx.enter_context(tc.tile_pool(name="pswp", bufs=2, space="PSUM"))

        xs = acts.tile([128, NK, SC], bf16)
        nc.sync.dma_start(xs, xT.rearrange("(ko ki) t -> ki ko t", ki=128))

        def qk_projection(w_dram, b_sb, g_sb, bg_sb, name):
            raw = acts.tile([128, NH, SC], bf16, tag="raw", name=f"raw_{name}")
            pss = pssp.tile([128, 512], f32, tag="pss", name=f"pss_{name}")
            for m in range(NH):
                wm = wstr.tile([128, NK, 128], bf16, tag="wm",
                               name=f"wm_{name}_{m}")
                nc.sync.dma_start(
                    wm, w_dram.rearrange(w_re, ki=128)[:, :, m * 128:(m + 1) * 128])
                ps = ppp.tile([128, 512], f32, tag="pp", name=f"pj_{name}_{m}")
                for kk in range(NK):
                    nc.tensor.matmul(
                        ps[:, :SC], wm[:, kk, :], xs[:, kk, :],
                        start=(kk == 0), stop=(kk == NK - 1))
                nc.scalar.activation(raw[:, m, :], ps[:, :SC], Act.Identity,
                                     bias=bg_sb[:, m:m + 1],
                                     scale=g_sb[:, m:m + 1])
                sq = sqp.tile([128, SC], f32, tag="sq")
                nc.scalar.activation(sq, ps[:, :SC], Act.Square,
                                     bias=b_sb[:, m:m + 1])
                nc.tensor.matmul(pss[0:1, :SC], ones_col, sq,
                                 start=(m == 0), stop=(m == NH - 1))
            r1 = smal.tile([1, SC], f32, tag="r1")
            nc.scalar.activation(r1, pss[0:1, :SC], Act.Sqrt,
                                 scale=constv(1.0 / DIM)[0:1],
                                 bias=constv(EPS)[0:1])
            rr = smal.tile([1, SC], f32, tag="rr")
            nc.vector.reciprocal(rr, r1)
            rrb = pswp.tile([128, 512], f32, tag="psw", name=f"rrb_{name}")
            nc.tensor.matmul(rrb[:, :SC], ones_row_f, rr, start=True, stop=True)
            ccr = csrp.tile([128, SC], f32, tag="ccr")
            ssr = csrp.tile([128, SC], f32, tag="ssr")
            nc.vector.tensor_mul(ccr, cc, rrb[:, :SC])
            nc.vector.tensor_mul(ssr, ss, rrb[:, :SC])
            return raw, ccr, ssr

        def rope_chunk(raw, ccr, ssr, m, dst_ap, name):
            # dst = raw*ccr + swap_halves(raw)*ssr   (swap via PE matmul)
            pw = pswp.tile([128, 512], f32, tag="psw", name=f"sw_{name}_{m}")
            nc.tensor.matmul(pw[:, :SC], psw_sb, raw[:, m, :],
                             start=True, stop=True)
            m1 = sqp.tile([128, SC], f32, tag="m1")
            nc.gpsimd.tensor_mul(m1, raw[:, m, :], ccr)
            m2 = sqp.tile([128, SC], f32, tag="m2")
            nc.vector.tensor_mul(m2, pw[:, :SC], ssr)
            nc.gpsimd.tensor_add(dst_ap, m1, m2)

        # ---------- K ----------
        raw_k, ccr_k, ssr_k = qk_projection(wk, bk_sb, gk_sb, bkg, "k")
        for m in range(NH):
            kn = knp.tile([128, SC], bf16, tag="kn", name=f"kn_{m}")
            rope_chunk(raw_k, ccr_k, ssr_k, m, kn, "k")
            nc.sync.dma_start(
                kv_cc_in[0, m].rearrange("(d t) -> d t", d=HD), kn)

        # ---------- V (direct [t, d] production, sequential tci) ----------
        vt = acts.tile([128, 3, DIM], bf16)
        for oc in range(3):
            wc = wstr.tile([128, NK, 512], bf16, tag="wc", name=f"wv_{oc}")
            nc.sync.dma_start(
                wc, wv.rearrange(w_re, ki=128)[:, :, oc * 512:(oc + 1) * 512])
            for tci in range(3):
                pv = ppp.tile([128, 512], f32, tag="pp", name=f"pv_{oc}_{tci}")
                for kk in range(NK):
                    nc.tensor.matmul(
                        pv[:ST, :], xs[:, kk, tci * ST:(tci + 1) * ST],
                        wc[:, kk, :], start=(kk == 0), stop=False)
                nc.tensor.matmul(
                    pv[:ST, :], ones_row[:, :ST],
                    bv_sb[:, oc * 512:(oc + 1) * 512],
                    start=False, stop=True)
                nc.vector.tensor_copy(
                    vt[:ST, tci, oc * 512:(oc + 1) * 512], pv[:ST, :])
        for h in range(NH):
            eng = nc.sync if h % 2 == 0 else nc.scalar
            eng.dma_start(
                kv_cc_in[1, h].rearrange("(tc p d) -> p tc d",
                                         tc=3, p=ST, d=HD),
                vt[:ST, :, h * HD:(h + 1) * HD])
        nc.gpsimd.collective_compute(
            "AllGather", Alu.bypass, replica_groups=rgroups,
            ins=[kv_cc_in.opt()], outs=[kvg.opt()])

        # ---------- Q ----------
        raw_q, ccr_q, ssr_q = qk_projection(wq, bq_sb, gq_sb, bqg, "q")
        qn = outp.tile([128, NH, SC], bf16)
        for m in range(NH):
            rope_chunk(raw_q, ccr_q, ssr_q, m, qn[:, m, :], "q")

        pctx.close()   # free xs/raw/vt SBUF + projection PSUM

        # ================= phase A: attention =============================
        wpool2 = ctx.enter_context(tc.tile_pool(name="wpool2", bufs=1))
        grp = ctx.enter_context(tc.tile_pool(name="grp", bufs=2, space="PSUM"))
        posp = ctx.enter_context(tc.tile_pool(name="posp", bufs=2, space="PSUM"))

        part1 = wpool2.tile([128, NH, 3, 130], f32)
        oT = wpool2.tile([128, NH, SC], bf16)
        smv = constv(sm_scale)

        def attn_pass(h, kt, vt_t, ntiles, phase):
            pos = posp.tile([128, 3, 130], f32, tag="pos",
                            name=f"pos_{phase}_{h}")
            # open the bank's single accumulation group (bank-wide bit clear)
            nc.tensor.matmul(pos[:, :, :], zero_col, zrow[:, 0:390],
                             start=True, stop=False, skip_group_check=True)
            for g0 in range(0, ntiles, 3):
                gs = min(3, ntiles - g0)
                gp = grp.tile([128, 3, 512], f32, tag="grp",
                              name=f"g_{phase}_{h}_{g0}")
                for j in range(gs):
                    lt = g0 + j
                    nc.tensor.matmul(
                        gp[:, j, :SC], kt[:, lt * 128:(lt + 1) * 128],
                        qn[:, h, :], start=True, stop=True)
                esc = escp.tile([128, 3, SC], bf16, tag="esc")
                nc.scalar.activation(esc[:, :gs, :], gp[:, :gs, :SC],
                                     Act.Exp, scale=smv)
                for j in range(gs):
                    lt = g0 + j
                    for si in range(3):
                        nc.tensor.matmul(
                            pos[:ST, si, 0:129],
                            esc[:, j, si * ST:(si + 1) * ST],
                            vt_t[:, lt, 0:129],
                            start=False,
                            stop=(lt == ntiles - 1 and si == 2),
                            skip_group_check=True)
            return pos

        # ---- pass 1: prior KV (overlaps the AllGather) ----
        for h in range(NH):
            pkh, pvh = (prior_tiles[h] if h in prior_tiles
                        else load_prior(h, False))
            pos = attn_pass(h, pkh, pvh, NTP, "p")
            nc.vector.tensor_copy(part1[:ST, h, :, 0:129], pos[:ST, :, 0:129])
        if _dbg:
            nc.sync.dma_start(dbg_part1, part1)

        # ---- pass 2: gathered current KV ----
        for h in range(NH):
            kch = kvp.tile([128, NCK], bf16, tag="kload", name=f"kc_{h}")
            for c in range(NC):
                nc.sync.dma_start(
                    kch[:, c * SC:(c + 1) * SC],
                    kvg[c, 0, h].rearrange("(p t) -> p t", p=HD))
            if NCK > S_TOTAL:
                nc.vector.memset(kch[:, S_TOTAL:NCK], 0.0)
            vch = kvp.tile([128, NTC, 130], bf16, tag="vload", name=f"vc_{h}")
            if NCK > S_TOTAL:
                # pad rows live in the last tile; zero it before the row DMAs
                nc.vector.memset(vch[:, NTC - 1, 0:HD], 0.0)
            for c in range(NC):
                r0 = c * SC
                src2d = kvg[c, 1, h].rearrange("(s d) -> s d", s=SC, d=HD)
                eng = nc.scalar if c < 3 else nc.sync
                for a, b in _segments(r0, r0 + SC):
                    eng.dma_start(
                        vch[a % 128:a % 128 + (b - a), a // 128, 0:HD],
                        src2d[a - r0:b - r0, :])
            nc.vector.memset(vch[:, :, 128:129], 1.0)
            if _dbg and h == 0:
                nc.sync.dma_start(dbg_kch, kch)
                nc.sync.dma_start(dbg_vch, vch)
            pos = attn_pass(h, kch, vch, NTC, "c")

            # finalize head: num/den merge, divide, transpose to [d, t]
            tmp = sqp.tile([128, 3, 130], f32, tag="tmp", name=f"tmp_{h}")
            nc.vector.tensor_add(tmp[:ST, :, 0:129], pos[:ST, :, 0:129],
                                 part1[:ST, h, :, 0:129])
            den = smal.tile([128, 3, 1], f32, tag="den")
            nc.vector.tensor_scalar_add(den[:ST, :, :], tmp[:ST, :, 128:129],
                                        -float(n_pads))
            rcp = smal.tile([128, 3, 1], f32, tag="rcp")
            nc.vector.reciprocal(rcp[:ST, :, :], den[:ST, :, :])
            odv = sqp.tile([128, 3, 128], f32, tag="odv", name=f"odv_{h}")
            for si in range(3):
                nc.vector.tensor_scalar_mul(odv[:ST, si, :],
                                            tmp[:ST, si, 0:128],
                                            rcp[:ST, si, 0:1])
            ptr = posp.tile([128, 512], f32, tag="pos", name=f"ptr_{h}")
            for si in range(3):
                nc.tensor.transpose(ptr[:, si * ST:(si + 1) * ST],
                                    odv[:ST, si, :], ident[:ST, :ST])
            nc.vector.tensor_copy(oT[:, h, :], ptr[:, 0:SC])

        # ---------- output projection (streamed wo, psum -> DRAM direct) --
        for oc in range(3):
            woc = wstr.tile([128, NK, 512], bf16, tag="wc", name=f"wo_{oc}")
            nc.sync.dma_start(
                woc, wo.rearrange(w_re, ki=128)[:, :, oc * 512:(oc + 1) * 512])
            for tci in range(3):
                po = posp.tile([128, 512], f32, tag="pos",
                               name=f"po_{oc}_{tci}")
                for hh in range(NH):
                    nc.tensor.matmul(
                        po[:ST, :], oT[:, hh, tci * ST:(tci + 1) * ST],
                        woc[:, hh, :], start=(hh == 0), stop=False)
                nc.tensor.matmul(
                    po[:ST, :], ones_row[:, :ST],
                    bo_sb[:, oc * 512:(oc + 1) * 512],
                    start=False, stop=True)
                ob = sqp.tile([128, 512], f32, tag="ob", name=f"ob_{oc}_{tci}")
                nc.vector.tensor_copy(ob[:ST, :], po[:ST, :])
                nc.sync.dma_start(
                    out[tci * ST:(tci + 1) * ST, oc * 512:(oc + 1) * 512],
                    ob[:ST, :])

    nc.compile()
    return nc


def _prep(inputs):
    x = np.asarray(inputs["x"], np.float32)
    freqs_angle = np.asarray(inputs["freqs_angle"], np.float32)
    prior_k = np.asarray(inputs["prior_k"], np.float32)
    prior_v = np.asarray(inputs["prior_v"], np.float32)
    cs = int(np.asarray(inputs["current_start"]))

    block = 3 * FRAME
    block_end = (cs // block + 1) * block
    keep_from = max(0, block_end - 6 * FRAME)
    keep_size = min(cs + S_TOTAL - keep_from, prior_k.shape[0] + S_TOTAL)
    n_prior = keep_size - S_TOTAL
    p0 = prior_k.shape[0] - n_prior
    npk = -(-n_prior // 128) * 128
    nck = -(-S_TOTAL // 128) * 128
    n_pads = (npk - n_prior) + (nck - S_TOTAL)

    perm = np.concatenate(
        [h * HD + np.concatenate([np.arange(0, HD, 2), np.arange(1, HD, 2)])
         for h in range(NH)])

    WqT = np.ascontiguousarray(np.asarray(inputs["Wq"], np.float32)[perm].T).astype(_BF16)
    WkT = np.ascontiguousarray(np.asarray(inputs["Wk"], np.float32)[perm].T).astype(_BF16)
    WvT = np.ascontiguousarray(np.asarray(inputs["Wv"], np.float32).T).astype(_BF16)
    WoT = np.ascontiguousarray(np.asarray(inputs["Wo"], np.float32).T).astype(_BF16)

    def two(vec, p=None):
        v = np.asarray(vec, np.float32)
        if p is not None:
            v = v[p]
        return np.ascontiguousarray(v.reshape(NH, HD).T)

    bq2 = two(inputs["bq"], perm)
    bk2 = two(inputs["bk"], perm)
    gq2 = two(inputs["gq"], perm)
    gk2 = two(inputs["gk"], perm)
    bv1 = np.asarray(inputs["bv"], np.float32).reshape(1, DIM).astype(_BF16)
    bo1 = np.asarray(inputs["bo"], np.float32).reshape(1, DIM).astype(_BF16)

    pswT = np.zeros((HD, HD), _BF16)
    for r in range(HD):
        pswT[(r + HALF) % HD, r] = 1.0   # lhsT of the half-swap permutation

    theta = _build_theta(freqs_angle, cs)          # [S, 64]
    thetaT = np.ascontiguousarray(theta.T)

    pk = prior_k[p0:].reshape(n_prior, DIM)[:, perm]
    priorKT = np.zeros((DIM, npk), np.float32)
    priorKT[:, :n_prior] = pk.T
    priorKT = np.ascontiguousarray(priorKT.reshape(NH, HD, npk)).astype(_BF16)

    ntp = npk // 128
    buf = np.zeros((npk, NH, HD), np.float32)
    buf[:n_prior] = prior_v[p0:]
    priorVT2 = np.ascontiguousarray(
        buf.reshape(ntp, 128, NH, HD).transpose(2, 1, 0, 3)).astype(_BF16)

    xT = np.ascontiguousarray(x[0].T).astype(_BF16)              # [DIM, S]

    shared = dict(wq=WqT, wk=WkT, wv=WvT, wo=WoT, bq2=bq2, bk2=bk2,
                  gq2=gq2, gk2=gk2, bv1=bv1, bo1=bo1, pswT=pswT,
                  priorKT=priorKT, priorVT2=priorVT2)
    in_maps = []
    for c in range(NC):
        m = dict(shared)
        m["xT"] = np.ascontiguousarray(xT[:, c * SC:(c + 1) * SC])
        m["thetaT"] = np.ascontiguousarray(thetaT[:, c * SC:(c + 1) * SC])
        in_maps.append(m)
    return in_maps, (n_prior, npk, n_pads)


def kernel(**inputs) -> np.ndarray:
    import os
    from concourse.bass_utils import run_bass_kernel_spmd

    in_maps, key = _prep(inputs)
    if key not in _cache:
        _cache[key] = _build_program(*key)
    nc = _cache[key]

    trace = bool(int(os.environ.get("KERNEL_TRACE", "0")))
    try:
        res = run_bass_kernel_spmd(
            nc, in_maps, core_ids=list(range(NC)), trace=trace,
            trace_cores=list(range(NC)) if trace else None)
    except ModuleNotFoundError:
        res = run_bass_kernel_spmd(nc, in_maps, core_ids=list(range(NC)),
                                   trace=False)
    kernel.last_results = res
    outp = np.concatenate([res.results[c]["out"] for c in range(NC)], axis=0)
    return outp[None].astype(np.float32)


# revision 16
# speedup vs baseline: 1.1239x; 1.0373x over previous
"""Trainium2 Bass kernel for nn_CausalSelfAttention_5411658793445.

Sharding: queries (token dim) split 8 ways; K/V projection also token-split,
with the current block's roped K / V exchanged via ONE merged compact
AllGather so every core attends over the full kept KV window.

v3 structure:
  - order: K proj -> V proj -> merged K+V AllGather -> Q proj -> pass-1
    attention over prior KV (overlaps the AllGather) -> pass-2 over gathered
    current KV -> output projection.
  - prior K/V for the first 8 heads prefetched at t~0 (before the AllGather
    floods the shared DMA engines); weights streamed per-head chunk.
  - attention: l-tiles in groups of 3; one batched ACT exp per group
    (3-bank PSUM tile).  AV numerator+denominator via fused [V | ones]
    129-wide rhs into one packed PSUM bank [128, 3si, 130].  A single
    zeroing matmul opens the bank's accumulation group (start=True clears
    has_written bank-wide, so per-si interleaved starts would corrupt
    each other); all AV matmuls then accumulate with start=False.
  - 21+21 l-tiles (96 zero pads) instead of 45.
  - DVE offload: vt/part1 copies + one rope multiply run on GpSimd (Pool).
"""

import math
from contextlib import ExitStack

import numpy as np
import ml_dtypes

NC = 8
DIM, NH, HD = 1536, 12, 128
HALF = 64
H, W = 22, 40
FRAME = H * W            # 880
S_TOTAL = 2640
SC = S_TOTAL // NC       # 330
ST = 110                 # s-subtile (330 = 3*110)
NK = DIM // 128          # 12 contraction chunks
EPS = 1e-6
CT = 22
CH = 21
CW = 21
PREFETCH = 7             # prior-KV head pairs prefetched before the AllGather

_BF16 = ml_dtypes.bfloat16
_cache: dict = {}


def _build_theta(freqs_angle, cs):
    start_frame = cs // FRAME
    nf = S_TOTAL // FRAME
    t = freqs_angle[start_frame:start_frame + nf, :CT]
    h = freqs_angle[:H, CT:CT + CH]
    w = freqs_angle[:W, CT + CH:CT + CH + CW]
    tf = np.broadcast_to(t[:, None, None, :], (nf, H, W, CT))
    hf = np.broadcast_to(h[None, :, None, :], (nf, H, W, CH))
    wf = np.broadcast_to(w[None, None, :, :], (nf, H, W, CW))
    return np.concatenate([tf, hf, wf], axis=-1).reshape(nf * H * W, HALF)


def _segments(r0, r1):
    """Split row range [r0, r1) at 128 boundaries -> (a, b) pieces."""
    a = r0
    while a < r1:
        b = min(r1, (a // 128 + 1) * 128)
        yield a, b
        a = b


def _build_program(n_prior, npk, n_pads):
    import concourse.bass as bass  # noqa: F401
    import concourse.tile as tile
    from concourse import bacc, mybir
    from concourse.masks import make_identity

    f32 = mybir.dt.float32
    bf16 = mybir.dt.bfloat16
    Act = mybir.ActivationFunctionType
    Alu = mybir.AluOpType

    NTP = npk // 128                  # prior l-tiles (21)
    NCK = -(-S_TOTAL // 128) * 128    # current cols padded (2688)
    NTC = NCK // 128                  # current l-tiles (21)
    sm_scale = 1.0 / math.sqrt(HD)

    nc = bacc.Bacc("TRN2", target_bir_lowering=False, debug=False,
                   num_devices=NC)

    xT = nc.dram_tensor("xT", [DIM, SC], bf16, kind="ExternalInput").ap()
    thetaT = nc.dram_tensor("thetaT", [HALF, SC], f32, kind="ExternalInput").ap()
    wq = nc.dram_tensor("wq", [DIM, DIM], bf16, kind="ExternalInput").ap()
    wk = nc.dram_tensor("wk", [DIM, DIM], bf16, kind="ExternalInput").ap()
    wv = nc.dram_tensor("wv", [DIM, DIM], bf16, kind="ExternalInput").ap()
    wo = nc.dram_tensor("wo", [DIM, DIM], bf16, kind="ExternalInput").ap()
    bq2 = nc.dram_tensor("bq2", [HD, NH], f32, kind="ExternalInput").ap()
    bk2 = nc.dram_tensor("bk2", [HD, NH], f32, kind="ExternalInput").ap()
    gq2 = nc.dram_tensor("gq2", [HD, NH], f32, kind="ExternalInput").ap()
    gk2 = nc.dram_tensor("gk2", [HD, NH], f32, kind="ExternalInput").ap()
    bv1 = nc.dram_tensor("bv1", [1, DIM], bf16, kind="ExternalInput").ap()
    bo1 = nc.dram_tensor("bo1", [1, DIM], bf16, kind="ExternalInput").ap()
    pswT = nc.dram_tensor("pswT", [HD, HD], bf16, kind="ExternalInput").ap()
    priorKT = nc.dram_tensor("priorKT", [NH, HD, npk], bf16,
                             kind="ExternalInput").ap()
    # host-pretiled prior V with baked ones col (128)=1, (129)=0
    priorVT2 = nc.dram_tensor("priorVT2", [NH, 128, NTP, 130], bf16,
                              kind="ExternalInput").ap()
    out = nc.dram_tensor("out", [SC, DIM], f32, kind="ExternalOutput").ap()
    import os as _os
    _dbg = bool(int(_os.environ.get("KERNEL_DEBUG", "0")))
    if _dbg:
        dbg_part1 = nc.dram_tensor("dbg_part1", [128, NH, 3, 130], f32,
                                   kind="ExternalOutput").ap()
        dbg_kch = nc.dram_tensor("dbg_kch", [128, NCK], bf16,
                                 kind="ExternalOutput").ap()
        dbg_vch = nc.dram_tensor("dbg_vch", [128, NTC, 130], bf16,
                                 kind="ExternalOutput").ap()

    w_re = "(ko ki) m -> ki ko m"

    with tile.TileContext(nc, trace_sim=False) as tc, ExitStack() as ctx:
        consts = ctx.enter_context(tc.tile_pool(name="consts", bufs=1))
        smal = ctx.enter_context(tc.tile_pool(name="smal", bufs=4))
        sqp = ctx.enter_context(tc.tile_pool(name="sqp", bufs=2))
        csrp = ctx.enter_context(tc.tile_pool(name="csrp", bufs=2))
        outp = ctx.enter_context(tc.tile_pool(name="outp", bufs=1))
        wstr = ctx.enter_context(tc.tile_pool(name="wstr", bufs=3))
        kvp = ctx.enter_context(tc.tile_pool(name="kvp", bufs=PREFETCH))
        escp = ctx.enter_context(tc.tile_pool(name="escp", bufs=3))
        dram = ctx.enter_context(tc.tile_pool(name="dram", bufs=1, space="DRAM"))

        # ---------- constants ----------
        _constv_cache = {}

        def constv(val):
            if val not in _constv_cache:
                t = consts.tile([128, 1], f32, name=f"cv_{len(_constv_cache)}")
                nc.vector.memset(t, val)
                _constv_cache[val] = t
            return _constv_cache[val]

        ident = consts.tile([128, 128], f32)
        make_identity(nc, ident)
        ones_col = consts.tile([128, 1], f32)
        nc.vector.memset(ones_col, 1.0)
        ones_row = consts.tile([1, 128], bf16)
        nc.vector.memset(ones_row, 1.0)
        ones_row_f = consts.tile([1, 128], f32)
        nc.vector.memset(ones_row_f, 1.0)
        zero_col = consts.tile([1, 128], bf16)
        nc.vector.memset(zero_col, 0.0)
        zrow = consts.tile([1, 512], bf16)
        nc.vector.memset(zrow, 1.0)
        psw_sb = consts.tile([HD, HD], bf16)
        nc.sync.dma_start(psw_sb, pswT)
        th2 = consts.tile([128, SC], f32)
        nc.sync.dma_start(th2[0:HALF, :], thetaT)
        nc.sync.dma_start(th2[HALF:128, :], thetaT)
        # CC = [cos; cos], SS = [-sin; sin]
        cc = consts.tile([128, SC], f32)
        ss = consts.tile([128, SC], f32)
        nc.scalar.activation(cc, th2, Act.Sin, bias=constv(math.pi / 2.0))
        nc.scalar.activation(ss[0:HALF, :], th2[0:HALF, :], Act.Sin,
                             scale=constv(-1.0)[0:HALF])
        nc.scalar.activation(ss[HALF:128, :], th2[HALF:128, :], Act.Sin)
        bq_sb = consts.tile([HD, NH], f32)
        bk_sb = consts.tile([HD, NH], f32)
        gq_sb = consts.tile([HD, NH], f32)
        gk_sb = consts.tile([HD, NH], f32)
        nc.sync.dma_start(bq_sb, bq2)
        nc.sync.dma_start(bk_sb, bk2)
        nc.sync.dma_start(gq_sb, gq2)
        nc.sync.dma_start(gk_sb, gk2)
        bqg = consts.tile([HD, NH], f32)
        bkg = consts.tile([HD, NH], f32)
        nc.vector.tensor_mul(bqg, bq_sb, gq_sb)
        nc.vector.tensor_mul(bkg, bk_sb, gk_sb)
        bv_sb = consts.tile([1, DIM], bf16)
        bo_sb = consts.tile([1, DIM], bf16)
        nc.sync.dma_start(bv_sb, bv1)
        nc.sync.dma_start(bo_sb, bo1)

        # ---------- internal DRAM for the merged collective ----------
        kv_cc_in = dram.tile([2, NH, SC * HD], bf16)
        kvg = dram.tile([NC, 2, NH, SC * HD], bf16, addr_space="Shared")
        rgroups = [list(range(NC))]

        # ---------- prior-KV prefetch (before any collective traffic) -----
        def load_prior(h, startup):
            pkh = kvp.tile([128, npk], bf16, tag="kload", name=f"pk_{h}")
            nc.sync.dma_start(pkh, priorKT[h])
            pvh = kvp.tile([128, NTP, 130], bf16, tag="vload", name=f"pv_{h}")
            nc.sync.dma_start(pvh, priorVT2[h])
            return pkh, pvh

        prior_tiles = {}
        for h in range(min(PREFETCH, NH)):
            prior_tiles[h] = load_prior(h, True)

        # ================= phase P: projections (inner pools) =============
        pctx = ExitStack()
        acts = pctx.enter_context(tc.tile_pool(name="acts", bufs=1))
        knp = pctx.enter_context(tc.tile_pool(name="knp", bufs=2))
        ppp = pctx.enter_context(tc.tile_pool(name="ppp", bufs=2, space="PSUM"))
        pssp = pctx.enter_context(tc.tile_pool(name="pssp", bufs=1, space="PSUM"))
        pswp = pctx.enter_context(tc.tile_pool(name="pswp", bufs=2, space="PSUM"))

        xs = acts.tile([128, NK, SC], bf16)
        nc.sync.dma_start(xs, xT.rearrange("(ko ki) t -> ki ko t", ki=128))

        def qk_projection(w_dram, b_sb, g_sb, bg_sb, name):
            raw = acts.tile([128, NH, SC], bf16, tag="raw", name=f"raw_{name}")
            pss = pssp.tile([128, 512], f32, tag="pss", name=f"pss_{name}")
            for m in range(NH):
                wm = wstr.tile([128, NK, 128], bf16, tag="wm",
                               name=f"wm_{name}_{m}")
                nc.sync.dma_start(
                    wm, w_dram.rearrange(w_re, ki=128)[:, :, m * 128:(m + 1) * 128])
                ps = ppp.tile([128, 512], f32, tag="pp", name=f"pj_{name}_{m}")
                for kk in range(NK):
                    nc.tensor.matmul(
                        ps[:, :SC], wm[:, kk, :], xs[:, kk, :],
                        start=(kk == 0), stop=(kk == NK - 1))
                nc.scalar.activation(raw[:, m, :], ps[:, :SC], Act.Identity,
                                     bias=bg_sb[:, m:m + 1],
                                     scale=g_sb[:, m:m + 1])
                sq = sqp.tile([128, SC], f32, tag="sq")
                nc.scalar.activation(sq, ps[:, :SC], Act.Square,
                                     bias=b_sb[:, m:m + 1])
                nc.tensor.matmul(pss[0:1, :SC], ones_col, sq,
                                 start=(m == 0), stop=(m == NH - 1))
            r1 = smal.tile([1, SC], f32, tag="r1")
            nc.scalar.activation(r1, pss[0:1, :SC], Act.Sqrt,
                                 scale=constv(1.0 / DIM)[0:1],
                                 bias=constv(EPS)[0:1])
            rr = smal.tile([1, SC], f32, tag="rr")
            nc.vector.reciprocal(rr, r1)
            rrb = pswp.tile([128, 512], f32, tag="psw", name=f"rrb_{name}")
            nc.tensor.matmul(rrb[:, :SC], ones_row_f, rr, start=True, stop=True)
            ccr = csrp.tile([128, SC], f32, tag="ccr")
            ssr = csrp.tile([128, SC], f32, tag="ssr")
            nc.vector.tensor_mul(ccr, cc, rrb[:, :SC])
            nc.vector.tensor_mul(ssr, ss, rrb[:, :SC])
            return raw, ccr, ssr

        def rope_chunk(raw, ccr, ssr, m, dst_ap, name, use_pool=True):
            # dst = raw*ccr + swap_halves(raw)*ssr   (swap via PE matmul)
            pw = pswp.tile([128, 512], f32, tag="psw", name=f"sw_{name}_{m}")
            nc.tensor.matmul(pw[:, :SC], psw_sb, raw[:, m, :],
                             start=True, stop=True)
            m1 = sqp.tile([128, SC], f32, tag="m1")
            (nc.gpsimd if use_pool else nc.vector).tensor_mul(
                m1, raw[:, m, :], ccr)
            m2 = sqp.tile([128, SC], f32, tag="m2")
            nc.vector.tensor_mul(m2, pw[:, :SC], ssr)
            (nc.gpsimd if use_pool else nc.vector).tensor_add(dst_ap, m1, m2)

        # ---------- K ----------
        raw_k, ccr_k, ssr_k = qk_projection(wk, bk_sb, gk_sb, bkg, "k")
        for m in range(NH):
            kn = knp.tile([128, SC], bf16, tag="kn", name=f"kn_{m}")
            rope_chunk(raw_k, ccr_k, ssr_k, m, kn, "k")
            nc.sync.dma_start(
                kv_cc_in[0, m].rearrange("(d t) -> d t", d=HD), kn)

        # ---------- V (direct [t, d] production, sequential tci) ----------
        vt = acts.tile([128, 3, DIM], bf16)
        for oc in range(3):
            wc = wstr.tile([128, NK, 512], bf16, tag="wc", name=f"wv_{oc}")
            nc.sync.dma_start(
                wc, wv.rearrange(w_re, ki=128)[:, :, oc * 512:(oc + 1) * 512])
            for tci in range(3):
                pv = ppp.tile([128, 512], f32, tag="pp", name=f"pv_{oc}_{tci}")
                for kk in range(NK):
                    nc.tensor.matmul(
                        pv[:ST, :], xs[:, kk, tci * ST:(tci + 1) * ST],
                        wc[:, kk, :], start=(kk == 0), stop=False)
                nc.tensor.matmul(
                    pv[:ST, :], ones_row[:, :ST],
                    bv_sb[:, oc * 512:(oc + 1) * 512],
                    start=False, stop=True)
                nc.vector.tensor_copy(
                    vt[:ST, tci, oc * 512:(oc + 1) * 512], pv[:ST, :])
        for h in range(NH):
            eng = nc.sync if h % 2 == 0 else nc.scalar
            eng.dma_start(
                kv_cc_in[1, h].rearrange("(tc p d) -> p tc d",
                                         tc=3, p=ST, d=HD),
                vt[:ST, :, h * HD:(h + 1) * HD])
        nc.gpsimd.collective_compute(
            "AllGather", Alu.bypass, replica_groups=rgroups,
            ins=[kv_cc_in.opt()], outs=[kvg.opt()])

        # ---------- Q ----------
        raw_q, ccr_q, ssr_q = qk_projection(wq, bq_sb, gq_sb, bqg, "q")
        qn = outp.tile([128, NH, SC], bf16)
        for m in range(NH):
            rope_chunk(raw_q, ccr_q, ssr_q, m, qn[:, m, :], "q", False)

        pctx.close()   # free xs/raw/vt SBUF + projection PSUM

        # ================= phase A: attention =============================
        wpool2 = ctx.enter_context(tc.tile_pool(name="wpool2", bufs=1))
        grp = ctx.enter_context(tc.tile_pool(name="grp", bufs=2, space="PSUM"))
        posp = ctx.enter_context(tc.tile_pool(name="posp", bufs=2, space="PSUM"))

        part1 = wpool2.tile([128, NH, 3, 130], f32)
        oT = wpool2.tile([128, NH, SC], bf16)
        smv = constv(sm_scale)

        def attn_pass(h, kt, vt_t, ntiles, phase):
            pos = posp.tile([128, 3, 130], f32, tag="pos",
                            name=f"pos_{phase}_{h}")
            # open the bank's single accumulation group (bank-wide bit clear)
            nc.tensor.matmul(pos[:, :, :], zero_col, zrow[:, 0:390],
                             start=True, stop=False, skip_group_check=True)
            for g0 in range(0, ntiles, 3):
                gs = min(3, ntiles - g0)
                gp = grp.tile([128, 3, 512], f32, tag="grp",
                              name=f"g_{phase}_{h}_{g0}")
                for j in range(gs):
                    lt = g0 + j
                    nc.tensor.matmul(
                        gp[:, j, :SC], kt[:, lt * 128:(lt + 1) * 128],
                        qn[:, h, :], start=True, stop=True)
                esc = escp.tile([128, 3, SC], bf16, tag="esc")
                nc.scalar.activation(esc[:, :gs, :], gp[:, :gs, :SC],
                                     Act.Exp, scale=smv)
                for j in range(gs):
                    lt = g0 + j
                    for si in range(3):
                        nc.tensor.matmul(
                            pos[:ST, si, 0:129],
                            esc[:, j, si * ST:(si + 1) * ST],
                            vt_t[:, lt, 0:129],
                            start=False,
                            stop=(lt == ntiles - 1 and si == 2),
                            skip_group_check=True)
            return pos

        # ---- pass 1: prior KV (overlaps the AllGather) ----
        for h in range(NH):
            pkh, pvh = (prior_tiles[h] if h in prior_tiles
                        else load_prior(h, False))
            pos = attn_pass(h, pkh, pvh, NTP, "p")
            nc.vector.tensor_copy(part1[:ST, h, :, 0:129], pos[:ST, :, 0:129])
        if _dbg:
            nc.sync.dma_start(dbg_part1, part1)

        # ---- pass 2: gathered current KV ----
        for h in range(NH):
            kch = kvp.tile([128, NCK], bf16, tag="kload", name=f"kc_{h}")
            nc.sync.dma_start(
                kch[:, 0:S_TOTAL].rearrange("p (c t) -> p c t", c=NC, t=SC),
                kvg[:, 0, h].rearrange("c (p t) -> p c t", p=HD, t=SC))
            if NCK > S_TOTAL:
                nc.vector.memset(kch[:, S_TOTAL:NCK], 0.0)
            vch = kvp.tile([128, NTC, 130], bf16, tag="vload", name=f"vc_{h}")
            if NCK > S_TOTAL:
                # pad rows live in the last tile; zero it before the row DMAs
                nc.vector.memset(vch[:, NTC - 1, 0:HD], 0.0)
            for c in range(NC):
                r0 = c * SC
                src2d = kvg[c, 1, h].rearrange("(s d) -> s d", s=SC, d=HD)
                eng = nc.scalar if c < 3 else nc.sync
                for a, b in _segments(r0, r0 + SC):
                    eng.dma_start(
                        vch[a % 128:a % 128 + (b - a), a // 128, 0:HD],
                        src2d[a - r0:b - r0, :])
            nc.vector.memset(vch[:, :, 128:129], 1.0)
            if _dbg and h == 0:
                nc.sync.dma_start(dbg_kch, kch)
                nc.sync.dma_start(dbg_vch, vch)
            pos = attn_pass(h, kch, vch, NTC, "c")

            # finalize head: num/den merge, divide, transpose to [d, t]
            tmp = sqp.tile([128, 3, 130], f32, tag="tmp", name=f"tmp_{h}")
            nc.vector.tensor_add(tmp[:ST, :, 0:129], pos[:ST, :, 0:129],
                                 part1[:ST, h, :, 0:129])
            den = smal.tile([128, 3, 1], f32, tag="den")
            nc.vector.tensor_scalar_add(den[:ST, :, :], tmp[:ST, :, 128:129],
                                        -float(n_pads))
            rcp = smal.tile([128, 3, 1], f32, tag="rcp")
            nc.vector.reciprocal(rcp[:ST, :, :], den[:ST, :, :])
            odv = sqp.tile([128, 3, 128], f32, tag="odv", name=f"odv_{h}")
            for si in range(3):
                nc.vector.tensor_scalar_mul(odv[:ST, si, :],
                                            tmp[:ST, si, 0:128],
                                            rcp[:ST, si, 0:1])
            ptr = posp.tile([128, 512], f32, tag="pos", name=f"ptr_{h}")
            for si in range(3):
                nc.tensor.transpose(ptr[:, si * ST:(si + 1) * ST],
                                    odv[:ST, si, :], ident[:ST, :ST])
            nc.vector.tensor_copy(oT[:, h, :], ptr[:, 0:SC])

        # ---------- output projection (streamed wo, psum -> DRAM direct) --
        for oc in range(3):
            woc = wstr.tile([128, NK, 512], bf16, tag="wc", name=f"wo_{oc}")
            nc.sync.dma_start(
                woc, wo.rearrange(w_re, ki=128)[:, :, oc * 512:(oc + 1) * 512])
            for tci in range(3):
                po = posp.tile([128, 512], f32, tag="pos",
                               name=f"po_{oc}_{tci}")
                for hh in range(NH):
                    nc.tensor.matmul(
                        po[:ST, :], oT[:, hh, tci * ST:(tci + 1) * ST],
                        woc[:, hh, :], start=(hh == 0), stop=False)
                nc.tensor.matmul(
                    po[:ST, :], ones_row[:, :ST],
                    bo_sb[:, oc * 512:(oc + 1) * 512],
                    start=False, stop=True)
                ob = sqp.tile([128, 512], f32, tag="ob", name=f"ob_{oc}_{tci}")
                nc.vector.tensor_copy(ob[:ST, :], po[:ST, :])
                nc.sync.dma_start(
                    out[tci * ST:(tci + 1) * ST, oc * 512:(oc + 1) * 512],
                    ob[:ST, :])

    nc.compile()
    return nc


def _prep(inputs):
    x = np.asarray(inputs["x"], np.float32)
    freqs_angle = np.asarray(inputs["freqs_angle"], np.float32)
    prior_k = np.asarray(inputs["prior_k"], np.float32)
    prior_v = np.asarray(inputs["prior_v"], np.float32)
    cs = int(np.asarray(inputs["current_start"]))

    block = 3 * FRAME
    block_end = (cs // block + 1) * block
    keep_from = max(0, block_end - 6 * FRAME)
    keep_size = min(cs + S_TOTAL - keep_from, prior_k.shape[0] + S_TOTAL)
    n_prior = keep_size - S_TOTAL
    p0 = prior_k.shape[0] - n_prior
    npk = -(-n_prior // 128) * 128
    nck = -(-S_TOTAL // 128) * 128
    n_pads = (npk - n_prior) + (nck - S_TOTAL)

    perm = np.concatenate(
        [h * HD + np.concatenate([np.arange(0, HD, 2), np.arange(1, HD, 2)])
         for h in range(NH)])

    WqT = np.ascontiguousarray(np.asarray(inputs["Wq"], np.float32)[perm].T).astype(_BF16)
    WkT = np.ascontiguousarray(np.asarray(inputs["Wk"], np.float32)[perm].T).astype(_BF16)
    WvT = np.ascontiguousarray(np.asarray(inputs["Wv"], np.float32).T).astype(_BF16)
    WoT = np.ascontiguousarray(np.asarray(inputs["Wo"], np.float32).T).astype(_BF16)

    def two(vec, p=None):
        v = np.asarray(vec, np.float32)
        if p is not None:
            v = v[p]
        return np.ascontiguousarray(v.reshape(NH, HD).T)

    bq2 = two(inputs["bq"], perm)
    bk2 = two(inputs["bk"], perm)
    gq2 = two(inputs["gq"], perm)
    gk2 = two(inputs["gk"], perm)
    bv1 = np.asarray(inputs["bv"], np.float32).reshape(1, DIM).astype(_BF16)
    bo1 = np.asarray(inputs["bo"], np.float32).reshape(1, DIM).astype(_BF16)

    pswT = np.zeros((HD, HD), _BF16)
    for r in range(HD):
        pswT[(r + HALF) % HD, r] = 1.0   # lhsT of the half-swap permutation

    theta = _build_theta(freqs_angle, cs)          # [S, 64]
    thetaT = np.ascontiguousarray(theta.T)

    pk = prior_k[p0:].reshape(n_prior, DIM)[:, perm]
    priorKT = np.zeros((DIM, npk), np.float32)
    priorKT[:, :n_prior] = pk.T
    priorKT = np.ascontiguousarray(priorKT.reshape(NH, HD, npk)).astype(_BF16)

    ntp = npk // 128
    buf = np.zeros((npk, NH, 130), np.float32)
    buf[:n_prior, :, :HD] = prior_v[p0:]
    buf[:, :, 128] = 1.0
    priorVT2 = np.ascontiguousarray(
        buf.reshape(ntp, 128, NH, 130).transpose(2, 1, 0, 3)).astype(_BF16)

    xT = np.ascontiguousarray(x[0].T).astype(_BF16)              # [DIM, S]

    shared = dict(wq=WqT, wk=WkT, wv=WvT, wo=WoT, bq2=bq2, bk2=bk2,
                  gq2=gq2, gk2=gk2, bv1=bv1, bo1=bo1, pswT=pswT,
                  priorKT=priorKT, priorVT2=priorVT2)
    in_maps = []
    for c in range(NC):
        m = dict(shared)
        m["xT"] = np.ascontiguousarray(xT[:, c * SC:(c + 1) * SC])
        m["thetaT"] = np.ascontiguousarray(thetaT[:, c * SC:(c + 1) * SC])
        in_maps.append(m)
    return in_maps, (n_prior, npk, n_pads)


def kernel(**inputs) -> np.ndarray:
    import os
    from concourse.bass_utils import run_bass_kernel_spmd

    in_maps, key = _prep(inputs)
    if key not in _cache:
        _cache[key] = _build_program(*key)
    nc = _cache[key]

    trace = bool(int(os.environ.get("KERNEL_TRACE", "0")))
    try:
        res = run_bass_kernel_spmd(
            nc, in_maps, core_ids=list(range(NC)), trace=trace,
            trace_cores=list(range(NC)) if trace else None)
    except ModuleNotFoundError:
        res = run_bass_kernel_spmd(nc, in_maps, core_ids=list(range(NC)),
                                   trace=False)
    kernel.last_results = res
    outp = np.concatenate([res.results[c]["out"] for c in range(NC)], axis=0)
    return outp[None].astype(np.float32)
